# revision 1
# baseline (speedup 1.0000x reference)
"""Trainium2 Bass kernel for nn_GCNModel_75874892251953 (2-layer SAGEConv GNN
+ fc head), distributed over 8 NeuronCores.

Strategy (hardcoded for N=50000 nodes, E=800000 edges, IN=64, HID=128):
 - Nodes (and their incoming edges) are range-sharded across 8 cores
   (6250 nodes/core); x is replicated so layer-1 message gathering is local.
 - Per core, edges are dst-sorted and packed into 128-edge tiles grouped by
   128-node chunks (host-side layout planning only; all FLOPs on device).
 - Layer-1 aggregation: indirect-DMA gather of x[src] rows (256B each) +
   segment-sum on the tensor engine via per-tile one-hot selection matrices
   built on the vector engine (is_equal against an iota matrix).
 - Layer-2 needs s[src] = (h1 @ W2l.T)[src] for every edge: s is exchanged
   via an AllGather collective, then a 4-byte indirect-DMA gather + the same
   one-hot segment-sum machinery.
 - fc1's [256, N] weight is sharded along N; partial z vectors are
   AllReduce'd and the tiny fc2 head is computed redundantly on every core.
"""
import numpy as np

# ---------------------------------------------------------------- config ---
NCORES = 8
N = 50000
IN = 64
HID = 128
LH = 256


class Cfg:
    def __init__(self, n_nodes, ncores=NCORES):
        assert n_nodes % ncores == 0
        self.N = n_nodes
        self.NC = n_nodes // ncores          # nodes per core
        self.CH = -(-self.NC // 128)         # 128-node chunks per core
        self.NCPAD = self.CH * 128
        self.SH = self.NCPAD + 8             # s-shard slots (tail zeros)


# --------------------------------------------------------------- planner ---
def plan(edge_index, cfg):
    src = np.asarray(edge_index[0], dtype=np.int64)
    dst = np.asarray(edge_index[1], dtype=np.int64)
    NC, CH = cfg.NC, cfg.CH
    owner = dst // NC

    cores = []
    maxtiles = np.zeros((NCORES, CH), dtype=np.int64)
    for c in range(NCORES):
        m = owner == c
        s_c = src[m]
        d_c = dst[m] - c * NC
        order = np.argsort(d_c, kind="stable")
        s_c, d_c = s_c[order].astype(np.int64), d_c[order]
        cnt = np.bincount(d_c // 128, minlength=CH)
        maxtiles[c] = (cnt + 127) // 128
        cores.append((s_c, d_c, cnt))

    H = max(int(maxtiles.max()), 1)
    T = CH * H
    L = T * 128

    lo_j = np.full(H, 1000, dtype=np.int64)
    hi_j = np.full(H, -1, dtype=np.int64)
    percore = []
    for c in range(NCORES):
        s_c, d_c, cnt = cores[c]
        srcpad = np.full(L, cfg.N, dtype=np.int64)   # pad -> zero row of x
        dstloc = np.full(L, -1000.0, dtype=np.float32)
        off = np.concatenate([[0], np.cumsum(cnt)])
        for k in range(CH):
            e0, e1 = off[k], off[k + 1]
            n = e1 - e0
            base = k * H * 128
            srcpad[base:base + n] = s_c[e0:e1]
            dl = (d_c[e0:e1] - 128 * k).astype(np.float32)
            dstloc[base:base + n] = dl
            for j in range((n + 127) // 128):
                seg = dl[j * 128:(j + 1) * 128]
                lo_j[j] = min(lo_j[j], int(seg.min()))
                hi_j[j] = max(hi_j[j], int(seg.max()))
        percore.append({"srcpad": srcpad, "d_c": d_c})

    w = np.zeros(H, dtype=np.int64)
    W = 0
    for j in range(1, H):
        if hi_j[j] < 0:
            continue
        w[j] = lo_j[j]
        W = max(W, int(hi_j[j] - lo_j[j] + 1))
    W = max(16, -(-W // 16) * 16)
    assert W <= 128, f"window W={W} > 128"
    w = np.minimum(w, 128 - W)
    w[0] = 0

    for c in range(NCORES):
        p = percore[c]
        srcpad = p["srcpad"]
        p["idx1"] = srcpad.reshape(T, 128).T.astype(np.int32).copy()
        o = srcpad // NC
        l = srcpad - o * NC
        flat = cfg.SH * o + (l % 128) * CH + l // 128
        flat[srcpad == cfg.N] = cfg.NCPAD
        p["idx2"] = flat.reshape(T, 128).T.astype(np.int32).copy()
        deg = np.bincount(p["d_c"], minlength=NC).astype(np.float32)
        invd = 1.0 / np.maximum(deg, 1.0)
        invd_pad = np.concatenate([invd, np.ones(cfg.NCPAD - NC, np.float32)])
        p["invrep"] = np.tile(invd_pad, (IN, 1)).copy()
        p["invw"] = invd_pad.reshape(CH, 128).T.copy()
    for c in range(NCORES):
        s_c, d_c, cnt = cores[c]
        dstloc = np.full(L, -1000.0, dtype=np.float32)
        off = np.concatenate([[0], np.cumsum(cnt)])
        for k in range(CH):
            e0, e1 = off[k], off[k + 1]
            base = k * H * 128
            dstloc[base:base + e1 - e0] = (d_c[e0:e1] - 128 * k).astype(
                np.float32)
        percore[c]["dstloc2d"] = dstloc.reshape(T, 128).T.copy()

    return {"H": H, "T": T, "W": int(W), "w": w.tolist(), "cores": percore}


# ----------------------------------------------------------- bass builder ---
def build_bass(cfg, pl, b2val=0.0, fc2bval=0.0, g1_chunks=4, g2_groups=3,
               debug_out=False):
    """Builds the SPMD bass module. Returns (nc, input-name list)."""
    import concourse.bacc as bacc
    import concourse.tile as tile
    import concourse.mybir as mybir
    from concourse import bass

    f32 = mybir.dt.float32
    i32 = mybir.dt.int32
    H, T, W, w = pl["H"], pl["T"], pl["W"], pl["w"]
    CH, NCPAD, SH = cfg.CH, cfg.NCPAD, cfg.SH

    nc = bacc.Bacc("TRN2", target_bir_lowering=False, debug=False,
                   num_devices=NCORES)

    def din(name, shape, dt=f32):
        return nc.dram_tensor(name, shape, dt, kind="ExternalInput")

    x_d = din("x_full", [cfg.N + 1, IN])
    idx1_d = din("idx1", [128, T], i32)
    idx2_d = din("idx2", [128, T], i32)
    dstloc_d = din("dstloc", [128, T])
    xT_d = din("xT", [IN, NCPAD])
    invrep_d = din("invrep", [IN, NCPAD])
    invw_d = din("invw", [128, CH])
    fc1T_d = din("fc1T", [NCPAD, LH])
    fc1bw_d = din("fc1bw", [128, 2])
    fc2w_d = din("fc2w", [128, 2])
    w1lT_d = din("w1lT", [IN, HID])
    w1rT_d = din("w1rT", [IN, HID])
    b1_d = din("b1", [HID, 1])
    w2p_d = din("w2pair", [HID, 2])
    iota_d = din("iota", [128, 128])
    out_d = nc.dram_tensor("out", [1, 1], f32, kind="ExternalOutput")
    if debug_out:
        dbg_s_d = nc.dram_tensor("dbg_s", [1, SH], f32, kind="ExternalOutput")
        dbg_sf_d = nc.dram_tensor("dbg_sf", [NCORES * SH, 1], f32,
                                  kind="ExternalOutput")
        dbg_q_d = nc.dram_tensor("dbg_q", [128, CH], f32,
                                 kind="ExternalOutput")
        dbg_v_d = nc.dram_tensor("dbg_v", [128, CH], f32,
                                 kind="ExternalOutput")
        dbg_z_d = nc.dram_tensor("dbg_z", [128, 2], f32,
                                 kind="ExternalOutput")
        dbg_h_d = nc.dram_tensor("dbg_h", [HID, 256], f32,
                                 kind="ExternalOutput")
        dbg_sv_d = nc.dram_tensor("dbg_sv", [128, T], f32,
                                  kind="ExternalOutput")

    RG = [list(range(NCORES))]
    G1 = g1_chunks
    n_g1 = -(-CH // G1)

    with tile.TileContext(nc) as tc:
        with (
            tc.tile_pool(name="const", bufs=1) as cpool,
            tc.tile_pool(name="big", bufs=1) as bigpool,
            tc.tile_pool(name="fc1w", bufs=1) as fc1pool,
            tc.tile_pool(name="gbuf", bufs=2) as gpool,
            tc.tile_pool(name="sbuf", bufs=3) as spool,
            tc.tile_pool(name="dram", bufs=1, space="DRAM") as dpool,
        ):
            # ---- persistent loads
            def load(pool, dram, shape, dt=f32):
                t = pool.tile(shape, dt, tag=dram.name + "_sb")
                nc.sync.dma_start(out=t[:], in_=dram.ap())
                return t

            idx1_sb = load(bigpool, idx1_d, [128, T], i32)
            idx2_sb = load(bigpool, idx2_d, [128, T], i32)
            dstloc_sb = load(bigpool, dstloc_d, [128, T])
            xT_sb = load(bigpool, xT_d, [IN, NCPAD])
            invrep_sb = load(bigpool, invrep_d, [IN, NCPAD])
            invw_sb = load(cpool, invw_d, [128, CH])
            fc1bw_sb = load(cpool, fc1bw_d, [128, 2])
            fc2w_sb = load(cpool, fc2w_d, [128, 2])
            w1lT_sb = load(cpool, w1lT_d, [IN, HID])
            w1rT_sb = load(cpool, w1rT_d, [IN, HID])
            b1_sb = load(cpool, b1_d, [HID, 1])
            w2p_sb = load(cpool, w2p_d, [HID, 2])
            iota_sb = load(cpool, iota_d, [128, 128])

            fc1_tiles = []
            for k in range(CH):
                t = fc1pool.tile([128, LH], f32, tag=f"fc1w{k}")
                nc.sync.dma_start(out=t[:],
                                  in_=fc1T_d.ap()[k * 128:(k + 1) * 128, :])
                fc1_tiles.append(t)

            h1T_sb = bigpool.tile([HID, NCPAD], f32, tag="h1T")
            srw_sb = bigpool.tile([128, 2 * CH], f32, tag="srw")
            sval_sb = bigpool.tile([128, T], f32, tag="sval")
            qw_sb = bigpool.tile([128, CH], f32, tag="qw")
            vw_sb = bigpool.tile([128, CH], f32, tag="vw")
            zar_sb = cpool.tile([128, 2], f32, tag="zar")
            zero_sb = cpool.tile([1, 8], f32, tag="zero8")
            nc.vector.memset(zero_sb[:], 0.0)
            id1_sb = cpool.tile([1, 1], f32, tag="id1")
            nc.vector.memset(id1_sb[:], 1.0)
            b2rep_sb = cpool.tile([128, 1], f32, tag="b2rep")
            nc.vector.memset(b2rep_sb[:], b2val)
            fc2b_sb = cpool.tile([1, 1], f32, tag="fc2brep")
            nc.vector.memset(fc2b_sb[:], fc2bval)
            pred_sb = cpool.tile([1, 1], f32, tag="pred")

            s_shard = dpool.tile([1, SH], f32)
            s_full = dpool.tile([NCORES * SH, 1], f32)
            zin_dr = dpool.tile([128, 2], f32)
            zout_dr = dpool.tile([128, 2], f32)

            # =================== PHASE A: layer 1 ===================
            with (
                tc.tile_pool(name="psA", bufs=2, space="PSUM") as psA,
                tc.tile_pool(name="psH", bufs=2, space="PSUM") as psH,
                tc.tile_pool(name="psSR", bufs=2, space="PSUM") as psSR,
                tc.tile_pool(name="Sp", bufs=4) as Spool,
                tc.tile_pool(name="aggp", bufs=2) as aggpool,
            ):
                for g in range(1):
                    for k in range(CH):
                        psum = psA.tile([IN, 128], f32, tag="psA")
                        for j in range(H):
                            t = k * H + j
                            gbuf = gpool.tile([128, IN], f32, tag="gb")
                            nc.gpsimd.indirect_dma_start(
                                out=gbuf[:], out_offset=None,
                                in_=x_d.ap(),
                                in_offset=bass.IndirectOffsetOnAxis(
                                    ap=idx1_sb[:, t:t + 1], axis=0))
                            if j == 0:
                                S = Spool.tile([128, 128], f32, tag="S")
                                nc.vector.tensor_scalar(
                                    out=S[:], in0=iota_sb[:],
                                    scalar1=dstloc_sb[:, t:t + 1],
                                    scalar2=None,
                                    op0=mybir.AluOpType.is_equal)
                                nc.tensor.matmul(
                                    out=psum[:], lhsT=gbuf[:],
                                    rhs=S[:], start=True, stop=(H == 1))
                            else:
                                wj = w[j]
                                S = Spool.tile([128, W], f32, tag="S")
                                nc.vector.tensor_scalar(
                                    out=S[:], in0=iota_sb[:, wj:wj + W],
                                    scalar1=dstloc_sb[:, t:t + 1],
                                    scalar2=None,
                                    op0=mybir.AluOpType.is_equal)
                                nc.tensor.matmul(
                                    out=psum[:, wj:wj + W],
                                    lhsT=gbuf[:], rhs=S[:],
                                    start=False, stop=(j == H - 1))
                        aggn = aggpool.tile([IN, 128], f32, tag="aggn")
                        nc.vector.tensor_tensor(
                            out=aggn[:], in0=psum[:],
                            in1=invrep_sb[:, k * 128:(k + 1) * 128],
                            op=mybir.AluOpType.mult)
                        ph = psH.tile([HID, 128], f32, tag="psH")
                        nc.tensor.matmul(out=ph[:], lhsT=w1lT_sb[:],
                                         rhs=aggn[:], start=True, stop=False)
                        nc.tensor.matmul(
                            out=ph[:], lhsT=w1rT_sb[:],
                            rhs=xT_sb[:, k * 128:(k + 1) * 128],
                            start=False, stop=True)
                        nc.scalar.activation(
                            out=h1T_sb[:, k * 128:(k + 1) * 128], in_=ph[:],
                            func=mybir.ActivationFunctionType.Relu,
                            bias=b1_sb[:, 0:1])
                        psr = psSR.tile([128, 2], f32, tag="psSR")
                        nc.tensor.matmul(
                            out=psr[:],
                            lhsT=h1T_sb[:, k * 128:(k + 1) * 128],
                            rhs=w2p_sb[:], start=True, stop=True)
                        nc.scalar.copy(out=srw_sb[:, 2 * k:2 * k + 2],
                                       in_=psr[:])

            # s -> DRAM shard (wrapped layout: node l at pos (l%128)*CH+l//128)
            nc.sync.dma_start(out=s_shard[0:1, 0:NCPAD],
                              in_=srw_sb[:, 0:2 * CH:2])
            nc.sync.dma_start(out=s_shard[0:1, NCPAD:SH], in_=zero_sb[:])

            if debug_out:
                nc.sync.dma_start(out=dbg_s_d.ap()[0:1, 0:NCPAD],
                                  in_=srw_sb[:, 0:2 * CH:2])
                nc.sync.dma_start(out=dbg_h_d.ap(),
                                  in_=h1T_sb[:, 0:256])

            # =================== PHASE B: exchange ===================
            nc.gpsimd.collective_compute(
                "AllGather", mybir.AluOpType.bypass, replica_groups=RG,
                ins=[s_shard[:].opt()], outs=[s_full[:].opt()])

            # =================== PHASE C: layer 2 ===================
            with (
                tc.tile_pool(name="psQ", bufs=2, space="PSUM") as psQ,
                tc.tile_pool(name="psT", bufs=2, space="PSUM") as psT,
                tc.tile_pool(name="psZ", bufs=1, space="PSUM") as psZ,
                tc.tile_pool(name="psP", bufs=1, space="PSUM") as psP,
                tc.tile_pool(name="Sp2", bufs=4) as Spool2,
                tc.tile_pool(name="qtmp", bufs=2) as qpool,
            ):
                for g in range(1):
                    for k in range(CH):
                        psq = psQ.tile([1, 128], f32, tag="psQ")
                        for j in range(H):
                            t = k * H + j
                            nc.gpsimd.indirect_dma_start(
                                out=sval_sb[:, t:t + 1], out_offset=None,
                                in_=s_full[:],
                                in_offset=bass.IndirectOffsetOnAxis(
                                    ap=idx2_sb[:, t:t + 1], axis=0))
                            if j == 0:
                                S = Spool2.tile([128, 128], f32, tag="S2")
                                nc.vector.tensor_scalar(
                                    out=S[:], in0=iota_sb[:],
                                    scalar1=dstloc_sb[:, t:t + 1],
                                    scalar2=None,
                                    op0=mybir.AluOpType.is_equal)
                                nc.tensor.matmul(
                                    out=psq[:], lhsT=sval_sb[:, t:t + 1],
                                    rhs=S[:],
                                    start=True, stop=(H == 1))
                            else:
                                wj = w[j]
                                S = Spool2.tile([128, W], f32, tag="S2")
                                nc.vector.tensor_scalar(
                                    out=S[:], in0=iota_sb[:, wj:wj + W],
                                    scalar1=dstloc_sb[:, t:t + 1],
                                    scalar2=None,
                                    op0=mybir.AluOpType.is_equal)
                                nc.tensor.matmul(
                                    out=psq[0:1, wj:wj + W],
                                    lhsT=sval_sb[:, t:t + 1], rhs=S[:],
                                    start=False, stop=(j == H - 1))
                        qtmp = qpool.tile([1, 128], f32, tag="qtmp")
                        nc.scalar.copy(out=qtmp[:], in_=psq[:])
                        pst = psT.tile([128, 1], f32, tag="psT")
                        nc.tensor.transpose(out=pst[:], in_=qtmp[:],
                                            identity=id1_sb[:])
                        nc.vector.tensor_copy(out=qw_sb[:, k:k + 1],
                                              in_=pst[:])

                if debug_out:
                    sf_sb = bigpool.tile([NCORES, SH], f32, tag="sf_sb")
                    nc.sync.dma_start(
                        out=sf_sb[:],
                        in_=s_full[:].rearrange("(a b) 1 -> a b", a=NCORES))
                    nc.sync.dma_start(
                        out=dbg_sf_d.ap().rearrange("(a b) 1 -> a b",
                                                    a=NCORES),
                        in_=sf_sb[:])
                    nc.sync.dma_start(out=dbg_q_d.ap(), in_=qw_sb[:])
                    nc.sync.dma_start(out=dbg_sv_d.ap(), in_=sval_sb[:])

                # v = relu(q*invd + r + b2)
                nc.vector.tensor_tensor(out=vw_sb[:], in0=qw_sb[:],
                                        in1=invw_sb[:],
                                        op=mybir.AluOpType.mult)
                nc.vector.tensor_tensor(out=vw_sb[:], in0=vw_sb[:],
                                        in1=srw_sb[:, 1:2 * CH:2],
                                        op=mybir.AluOpType.add)
                nc.scalar.activation(out=vw_sb[:], in_=vw_sb[:],
                                     func=mybir.ActivationFunctionType.Relu,
                                     bias=b2rep_sb[:, 0:1])

                # fc1 partial: z[m] = sum_k fc1T[k][:,m].T @ v_w[:,k]
                pz0 = psZ.tile([128, 1], f32, tag="pz0")
                pz1 = psZ.tile([128, 1], f32, tag="pz1")
                for k in range(CH):
                    nc.tensor.matmul(out=pz0[:], lhsT=fc1_tiles[k][:, 0:128],
                                     rhs=vw_sb[:, k:k + 1],
                                     start=(k == 0), stop=(k == CH - 1))
                    nc.tensor.matmul(out=pz1[:], lhsT=fc1_tiles[k][:, 128:LH],
                                     rhs=vw_sb[:, k:k + 1],
                                     start=(k == 0), stop=(k == CH - 1))
                zf_sb = cpool.tile([128, 2], f32, tag="zf")
                nc.scalar.copy(out=zf_sb[:, 0:1], in_=pz0[:])
                nc.scalar.copy(out=zf_sb[:, 1:2], in_=pz1[:])
                nc.sync.dma_start(out=zin_dr[:], in_=zf_sb[:])
                if debug_out:
                    nc.sync.dma_start(out=dbg_v_d.ap(), in_=vw_sb[:])
                    nc.sync.dma_start(out=dbg_z_d.ap(), in_=zf_sb[:])
                nc.gpsimd.collective_compute(
                    "AllReduce", mybir.AluOpType.add, replica_groups=RG,
                    ins=[zin_dr[:].opt()], outs=[zout_dr[:].opt()])
                nc.sync.dma_start(out=zar_sb[:], in_=zout_dr[:])
                nc.vector.tensor_tensor(out=zar_sb[:], in0=zar_sb[:],
                                        in1=fc1bw_sb[:],
                                        op=mybir.AluOpType.add)
                pp = psP.tile([1, 1], f32, tag="pp")
                nc.tensor.matmul(out=pp[:], lhsT=zar_sb[:, 0:1],
                                 rhs=fc2w_sb[:, 0:1], start=True, stop=False)
                nc.tensor.matmul(out=pp[:], lhsT=zar_sb[:, 1:2],
                                 rhs=fc2w_sb[:, 1:2], start=False, stop=True)
                nc.scalar.copy(out=pred_sb[:], in_=pp[:])
                nc.vector.tensor_tensor(out=pred_sb[:], in0=pred_sb[:],
                                        in1=fc2b_sb[:],
                                        op=mybir.AluOpType.add)
                nc.sync.dma_start(out=out_d.ap(), in_=pred_sb[:])

    nc.compile()
    return nc


# ------------------------------------------------------------- host glue ---
def make_in_maps(cfg, pl, inputs):
    x = np.ascontiguousarray(np.asarray(inputs["x"], np.float32))
    W1l = np.asarray(inputs["W1l"], np.float32)
    b1l = np.asarray(inputs["b1l"], np.float32)
    W1r = np.asarray(inputs["W1r"], np.float32)
    W2l = np.asarray(inputs["W2l"], np.float32)
    W2r = np.asarray(inputs["W2r"], np.float32)
    fc1_W = np.asarray(inputs["fc1_W"], np.float32)
    fc1_b = np.asarray(inputs["fc1_b"], np.float32)
    fc2_W = np.asarray(inputs["fc2_W"], np.float32)
    b2l = np.asarray(inputs["b2l"], np.float32)
    fc2_b = np.asarray(inputs["fc2_b"], np.float32)
    NC, CH, NCPAD = cfg.NC, cfg.CH, cfg.NCPAD

    xpad = np.concatenate([x, np.zeros((1, IN), np.float32)], axis=0)
    iota = np.tile(np.arange(128, dtype=np.float32), (128, 1))
    in_maps = []
    for c in range(NCORES):
        p = pl["cores"][c]
        xc = x[c * NC:(c + 1) * NC]
        xT = np.zeros((IN, NCPAD), np.float32)
        xT[:, :NC] = xc.T
        fc1T = np.zeros((NCPAD, LH), np.float32)
        fc1T[:NC] = fc1_W[:, c * NC:(c + 1) * NC].T
        in_maps.append({
            "x_full": xpad,
            "idx1": p["idx1"], "idx2": p["idx2"],
            "dstloc": p["dstloc2d"],
            "xT": np.ascontiguousarray(xT),
            "invrep": np.ascontiguousarray(p["invrep"]),
            "invw": np.ascontiguousarray(p["invw"]),
            "fc1T": np.ascontiguousarray(fc1T),
            "fc1bw": np.ascontiguousarray(fc1_b.reshape(2, 128).T),
            "fc2w": np.ascontiguousarray(fc2_W[0].reshape(2, 128).T),
            "w1lT": np.ascontiguousarray(W1l.T),
            "w1rT": np.ascontiguousarray(W1r.T),
            "b1": np.ascontiguousarray(b1l.reshape(HID, 1)),
            "w2pair": np.ascontiguousarray(
                np.stack([W2l[0], W2r[0]], axis=1)),
            "iota": np.ascontiguousarray(iota),
        })
    return in_maps


def kernel(**inputs) -> np.ndarray:
    from concourse.bass_utils import run_bass_kernel_spmd
    cfg = Cfg(N)
    pl = plan(np.asarray(inputs["edge_index"]), cfg)
    nc = build_bass(cfg, pl,
                    b2val=float(np.asarray(inputs["b2l"]).reshape(-1)[0]),
                    fc2bval=float(np.asarray(inputs["fc2_b"]).reshape(-1)[0]))
    in_maps = make_in_maps(cfg, pl, inputs)
    res = run_bass_kernel_spmd(nc, in_maps, core_ids=list(range(NCORES)))
    pred = np.asarray(res.results[0]["out"], np.float32).reshape(())
    return pred



# revision 2
# speedup vs baseline: 3.1101x; 3.1101x over previous
"""Trainium2 Bass kernel for nn_GCNModel_75874892251953 (2-layer SAGEConv GNN
+ fc head), distributed over 8 NeuronCores.

The axon host->device tunnel (~36 MB/s, shared across all 8 cores) dominates
wall time, so the kernel is built around minimizing uploaded bytes:
 - x is uploaded SHARDED (bf16, 800KB/core) and AllGather'ed on device into
   a full DRAM copy that layer-1 edge gathers read from.
 - fc1's [256, N] weight is sharded along N and shipped in bf16.
 - Edge metadata per core: src as uint16 (cast to int32 on device), local
   dst slot as int8 (cast to f32 on device). No second index array: s is
   written to DRAM in linear node order so the layer-2 gather reuses src.
 - invd (1/deg) is shipped once per node ([1, NCPAD] f32) and broadcast
   across partitions on device; xT is produced by on-device PE transposes.

Compute per core (nodes range-sharded, edges owned by dst):
 - layer-1 aggregation: per 128-edge tile, indirect-DMA gather of x[src]
   rows + one-hot segment-sum matmul into a [64, 128] PSUM per dst chunk.
 - h1 = relu(W1l@agg + W1r@xT + b1) per chunk; s,r = h1.T @ [W2l|W2r].
 - s shards AllGather'ed ([1, 6250] f32 -> [50000, 1]); layer-2 per tile:
   4B indirect gather of s[src], SD = onehot(dst)*s_e, matmul(SD, ones)
   accumulating the whole core's q in one [128, CH] PSUM region.
 - v = relu(q*invd + r + b2); fc1 partials per chunk; AllReduce z; fc2.
"""
import numpy as np

NCORES = 8
N = 50000
IN = 64
HID = 128
LH = 256
NC = N // NCORES          # 6250 nodes per core
CH = -(-NC // 128)        # 49 chunks of 128 dst slots
NCPAD = CH * 128          # 6272
NCL = NC - 128 * (CH - 1)  # rows in the last (partial) chunk = 106


# --------------------------------------------------------------- planner ---
def plan(edge_index):
    src = np.asarray(edge_index[0], dtype=np.int64)
    dst = np.asarray(edge_index[1], dtype=np.int64)
    owner = dst // NC

    cores = []
    H = 1
    for c in range(NCORES):
        m = owner == c
        s_c = src[m]
        d_c = dst[m] - c * NC
        order = np.argsort(d_c, kind="stable")
        s_c, d_c = s_c[order], d_c[order]
        cnt = np.bincount(d_c // 128, minlength=CH)
        H = max(H, int(((cnt + 127) // 128).max()))
        deg = np.bincount(d_c, minlength=NC).astype(np.float32)
        cores.append((s_c, d_c, cnt, deg))

    T = CH * H
    L = T * 128
    percore = []
    for c in range(NCORES):
        s_c, d_c, cnt, deg = cores[c]
        srcpad = np.zeros(L, dtype=np.int64)          # pad -> node 0 (masked)
        dstl = np.full(L, -100, dtype=np.int64)       # pad -> no dst slot
        off = np.concatenate([[0], np.cumsum(cnt)])
        for k in range(CH):
            e0, e1 = off[k], off[k + 1]
            n = e1 - e0
            base = k * H * 128
            srcpad[base:base + n] = s_c[e0:e1]
            dstl[base:base + n] = d_c[e0:e1] - 128 * k
        invd = (1.0 / np.maximum(deg, 1.0)).astype(np.float32)
        invrow = np.concatenate([invd, np.ones(NCPAD - NC, np.float32)])
        percore.append({
            "idxs": srcpad.reshape(T, 128).T.astype(np.uint16).copy(),
            "dst8": dstl.reshape(T, 128).T.astype(np.int8).copy(),
            "invrow": invrow.reshape(1, NCPAD).copy(),
            "invw": invrow.reshape(CH, 128).T.copy(),
        })
    return {"H": H, "T": T, "cores": percore}


# ----------------------------------------------------------- bass builder ---
def build_bass(H, b2val=0.0, fc2bval=0.0, debug_out=False):
    import concourse.bacc as bacc
    import concourse.tile as tile
    import concourse.mybir as mybir
    from concourse import bass

    f32 = mybir.dt.float32
    bf16 = mybir.dt.bfloat16
    i32 = mybir.dt.int32
    u16 = mybir.dt.uint16
    i8 = mybir.dt.int8
    T = CH * H

    nc = bacc.Bacc("TRN2", target_bir_lowering=False, debug=False,
                   num_devices=NCORES)

    def din(name, shape, dt=f32):
        return nc.dram_tensor(name, shape, dt, kind="ExternalInput")

    x_in_d = din("x_in", [NC, IN], bf16)
    idx_d = din("idxs", [128, T], u16)
    dst8_d = din("dst8", [128, T], i8)
    invrow_d = din("invrow", [1, NCPAD])
    invw_d = din("invw", [128, CH])
    fc1T_d = din("fc1T", [NCPAD, LH], bf16)
    w1lT_d = din("w1lT", [IN, HID], bf16)
    w1rT_d = din("w1rT", [IN, HID], bf16)
    b1_d = din("b1", [HID, 1])
    w2p_d = din("w2pair", [HID, 2], bf16)
    fc1bw_d = din("fc1bw", [128, 2])
    fc2w_d = din("fc2w", [128, 2])
    iota_d = din("iota", [128, 128])
    idb_d = din("id128b", [128, 128], bf16)
    idf_d = din("id128f", [128, 128])
    out_d = nc.dram_tensor("out", [1, 1], f32, kind="ExternalOutput")
    if debug_out:
        dbg_s_d = nc.dram_tensor("dbg_s", [1, NC], f32, kind="ExternalOutput")
        dbg_v_d = nc.dram_tensor("dbg_v", [128, CH], f32,
                                 kind="ExternalOutput")
        dbg_z_d = nc.dram_tensor("dbg_z", [128, 2], f32,
                                 kind="ExternalOutput")

    RG = [list(range(NCORES))]

    with tile.TileContext(nc) as tc:
        with (
            tc.tile_pool(name="const", bufs=1) as cpool,
            tc.tile_pool(name="big", bufs=1) as bigpool,
            tc.tile_pool(name="fc1w", bufs=1) as fc1pool,
            tc.tile_pool(name="gbuf", bufs=4) as gpool,
            tc.tile_pool(name="dram", bufs=1, space="DRAM") as dpool,
        ):
            def load(pool, dram, shape, dt=f32):
                t = pool.tile(shape, dt, tag=dram.name + "_sb")
                nc.sync.dma_start(out=t[:], in_=dram.ap())
                return t

            idxu_sb = load(bigpool, idx_d, [128, T], u16)
            dst8_sb = load(bigpool, dst8_d, [128, T], i8)
            invw_sb = load(cpool, invw_d, [128, CH])
            invrow_sb = load(cpool, invrow_d, [1, NCPAD])
            iota_sb = load(cpool, iota_d, [128, 128])
            idb_sb = load(cpool, idb_d, [128, 128], bf16)
            idf_sb = load(cpool, idf_d, [128, 128])
            w1lT_sb = load(cpool, w1lT_d, [IN, HID], bf16)
            w1rT_sb = load(cpool, w1rT_d, [IN, HID], bf16)
            b1_sb = load(cpool, b1_d, [HID, 1])
            w2p_sb = load(cpool, w2p_d, [HID, 2], bf16)
            fc1bw_sb = load(cpool, fc1bw_d, [128, 2])
            fc2w_sb = load(cpool, fc2w_d, [128, 2])

            fc1_tiles = []
            for k in range(CH):
                t = fc1pool.tile([128, LH], bf16, tag=f"fc1w{k}")
                nc.sync.dma_start(out=t[:],
                                  in_=fc1T_d.ap()[k * 128:(k + 1) * 128, :])
                fc1_tiles.append(t)

            # device-side casts of edge metadata
            idx_sb = bigpool.tile([128, T], i32, tag="idx_i32")
            nc.vector.tensor_copy(out=idx_sb[:], in_=idxu_sb[:])
            dstf_sb = bigpool.tile([128, T], f32, tag="dst_f32")
            nc.vector.tensor_copy(out=dstf_sb[:], in_=dst8_sb[:])

            # invrep[f, n] = invd[n] on 64 partitions
            invrep_sb = bigpool.tile([IN, NCPAD], f32, tag="invrep")
            nc.gpsimd.partition_broadcast(invrep_sb[:], invrow_sb[:])

            srw_sb = bigpool.tile([128, 2 * CH], f32, tag="srw")
            ones_sb = cpool.tile([128, 1], f32, tag="ones")
            nc.vector.memset(ones_sb[:], 1.0)
            b2rep_sb = cpool.tile([128, 1], f32, tag="b2rep")
            nc.vector.memset(b2rep_sb[:], b2val)
            fc2b_sb = cpool.tile([1, 1], f32, tag="fc2brep")
            nc.vector.memset(fc2b_sb[:], fc2bval)
            pred_sb = cpool.tile([1, 1], f32, tag="pred")
            zar_sb = cpool.tile([128, 2], f32, tag="zar")
            zf_sb = cpool.tile([128, 2], f32, tag="zf")

            # stage own x shard to DRAM flat, AllGather to full copy
            xstage_sb = bigpool.tile([128, NC * IN // 128], bf16, tag="xstg")
            nc.sync.dma_start(out=xstage_sb[:], in_=x_in_d.ap())
            x_stage = dpool.tile([1, NC * IN], bf16)
            nc.sync.dma_start(out=x_stage[:], in_=xstage_sb[:])
            x_full = dpool.tile([NCORES * NC * IN, 1], bf16)
            s_shard = dpool.tile([1, NC], f32)
            s_full = dpool.tile([N, 1], f32)
            zin_dr = dpool.tile([128, 2], f32)
            zout_dr = dpool.tile([128, 2], f32)

            nc.gpsimd.collective_compute(
                "AllGather", mybir.AluOpType.bypass, replica_groups=RG,
                ins=[x_stage[:].opt()], outs=[x_full[:].opt()])
            x_rows = x_full[:].rearrange("(n f) 1 -> n f", f=IN)

            # =================== PHASE A: layer 1 ===================
            with (
                tc.tile_pool(name="psA", bufs=2, space="PSUM") as psA,
                tc.tile_pool(name="psH", bufs=2, space="PSUM") as psH,
                tc.tile_pool(name="psXT", bufs=1, space="PSUM") as psXT,
                tc.tile_pool(name="psSR", bufs=1, space="PSUM") as psSR,
                tc.tile_pool(name="psST", bufs=1, space="PSUM") as psST,
                tc.tile_pool(name="Sp", bufs=4) as Spool,
                tc.tile_pool(name="aggp", bufs=2) as aggpool,
                tc.tile_pool(name="xcp", bufs=2) as xcpool,
                tc.tile_pool(name="h1p", bufs=2) as h1pool,
            ):
                for k in range(CH):
                    psum = psA.tile([IN, 128], f32, tag="psA")
                    for j in range(H):
                        t = k * H + j
                        gbuf = gpool.tile([128, IN], bf16, tag="gb")
                        nc.gpsimd.indirect_dma_start(
                            out=gbuf[:], out_offset=None,
                            in_=x_rows,
                            in_offset=bass.IndirectOffsetOnAxis(
                                ap=idx_sb[:, t:t + 1], axis=0))
                        S = Spool.tile([128, 128], bf16, tag="S")
                        nc.vector.tensor_scalar(
                            out=S[:], in0=iota_sb[:],
                            scalar1=dstf_sb[:, t:t + 1], scalar2=None,
                            op0=mybir.AluOpType.is_equal)
                        nc.tensor.matmul(
                            out=psum[:], lhsT=gbuf[:], rhs=S[:],
                            start=(j == 0), stop=(j == H - 1))
                    aggn = aggpool.tile([IN, 128], bf16, tag="aggn")
                    nc.vector.tensor_tensor(
                        out=aggn[:], in0=psum[:],
                        in1=invrep_sb[:, k * 128:(k + 1) * 128],
                        op=mybir.AluOpType.mult)
                    xc = xcpool.tile([128, IN], bf16, tag="xc")
                    if k == CH - 1:
                        nc.vector.memset(xc[:], 0.0)
                        nc.sync.dma_start(
                            out=xc[0:NCL, :],
                            in_=x_in_d.ap()[k * 128:k * 128 + NCL, :])
                    else:
                        nc.sync.dma_start(
                            out=xc[:],
                            in_=x_in_d.ap()[k * 128:(k + 1) * 128, :])
                    pxT = psXT.tile([IN, 128], bf16, tag="pxT")
                    nc.tensor.transpose(out=pxT[:], in_=xc[:],
                                        identity=idb_sb[:])
                    xT = xcpool.tile([IN, 128], bf16, tag="xT")
                    nc.vector.tensor_copy(out=xT[:], in_=pxT[:])
                    ph = psH.tile([HID, 128], f32, tag="psH")
                    nc.tensor.matmul(out=ph[:], lhsT=w1lT_sb[:],
                                     rhs=aggn[:], start=True, stop=False)
                    nc.tensor.matmul(out=ph[:], lhsT=w1rT_sb[:],
                                     rhs=xT[:], start=False, stop=True)
                    h1c = h1pool.tile([HID, 128], bf16, tag="h1c")
                    nc.scalar.activation(
                        out=h1c[:], in_=ph[:],
                        func=mybir.ActivationFunctionType.Relu,
                        bias=b1_sb[:, 0:1])
                    psr = psSR.tile([128, 2], f32, tag="psSR")
                    nc.tensor.matmul(out=psr[:], lhsT=h1c[:], rhs=w2p_sb[:],
                                     start=True, stop=True)
                    nc.scalar.copy(out=srw_sb[:, 2 * k:2 * k + 2],
                                   in_=psr[:])

                # s (even cols of srw) -> linear node order in DRAM
                pst = psST.tile([CH, 128], f32, tag="pst")
                nc.tensor.transpose(out=pst[:], in_=srw_sb[:, 0:2 * CH:2],
                                    identity=idf_sb[:])
                s_cm = bigpool.tile([CH, 128], f32, tag="s_cm")
                nc.vector.tensor_copy(out=s_cm[:], in_=pst[:])
                nc.sync.dma_start(out=s_shard[0:1, 0:128 * (CH - 1)],
                                  in_=s_cm[0:CH - 1, :])
                nc.sync.dma_start(out=s_shard[0:1, 128 * (CH - 1):NC],
                                  in_=s_cm[CH - 1:CH, 0:NCL])

            if debug_out:
                nc.sync.dma_start(out=dbg_s_d.ap(), in_=s_shard[:])

            # =================== PHASE B: exchange ===================
            nc.gpsimd.collective_compute(
                "AllGather", mybir.AluOpType.bypass, replica_groups=RG,
                ins=[s_shard[:].opt()], outs=[s_full[:].opt()])

            # =================== PHASE C: layer 2 + head ===================
            with (
                tc.tile_pool(name="psQ", bufs=1, space="PSUM") as psQ,
                tc.tile_pool(name="psZ", bufs=1, space="PSUM") as psZ,
                tc.tile_pool(name="psP", bufs=1, space="PSUM") as psP,
                tc.tile_pool(name="Sp2", bufs=4) as Spool2,
                tc.tile_pool(name="svp", bufs=4) as svpool,
            ):
                qps = psQ.tile([128, CH], f32, tag="qps")
                for k in range(CH):
                    for j in range(H):
                        t = k * H + j
                        sv = svpool.tile([128, 1], f32, tag="sv")
                        nc.gpsimd.indirect_dma_start(
                            out=sv[:], out_offset=None,
                            in_=s_full[:],
                            in_offset=bass.IndirectOffsetOnAxis(
                                ap=idx_sb[:, t:t + 1], axis=0))
                        SD = Spool2.tile([128, 128], f32, tag="SD")
                        nc.vector.tensor_scalar(
                            out=SD[:], in0=iota_sb[:],
                            scalar1=dstf_sb[:, t:t + 1],
                            scalar2=sv[:, 0:1],
                            op0=mybir.AluOpType.is_equal,
                            op1=mybir.AluOpType.mult)
                        nc.tensor.matmul(
                            out=qps[:, k:k + 1], lhsT=SD[:], rhs=ones_sb[:],
                            start=(j == 0), stop=(j == H - 1))

                vw_sb = cpool.tile([128, CH], f32, tag="vw")
                nc.vector.tensor_tensor(out=vw_sb[:], in0=qps[:],
                                        in1=invw_sb[:],
                                        op=mybir.AluOpType.mult)
                nc.vector.tensor_tensor(out=vw_sb[:], in0=vw_sb[:],
                                        in1=srw_sb[:, 1:2 * CH:2],
                                        op=mybir.AluOpType.add)
                nc.scalar.activation(out=vw_sb[:], in_=vw_sb[:],
                                     func=mybir.ActivationFunctionType.Relu,
                                     bias=b2rep_sb[:, 0:1])
                vbf_sb = cpool.tile([128, CH], bf16, tag="vbf")
                nc.vector.tensor_copy(out=vbf_sb[:], in_=vw_sb[:])

                pz0 = psZ.tile([128, 1], f32, tag="pz0")
                pz1 = psZ.tile([128, 1], f32, tag="pz1")
                for k in range(CH):
                    nc.tensor.matmul(out=pz0[:],
                                     lhsT=fc1_tiles[k][:, 0:128],
                                     rhs=vbf_sb[:, k:k + 1],
                                     start=(k == 0), stop=(k == CH - 1))
                    nc.tensor.matmul(out=pz1[:],
                                     lhsT=fc1_tiles[k][:, 128:LH],
                                     rhs=vbf_sb[:, k:k + 1],
                                     start=(k == 0), stop=(k == CH - 1))
                nc.scalar.copy(out=zf_sb[:, 0:1], in_=pz0[:])
                nc.scalar.copy(out=zf_sb[:, 1:2], in_=pz1[:])
                nc.sync.dma_start(out=zin_dr[:], in_=zf_sb[:])
                if debug_out:
                    nc.sync.dma_start(out=dbg_v_d.ap(), in_=vw_sb[:])
                    nc.sync.dma_start(out=dbg_z_d.ap(), in_=zf_sb[:])
                nc.gpsimd.collective_compute(
                    "AllReduce", mybir.AluOpType.add, replica_groups=RG,
                    ins=[zin_dr[:].opt()], outs=[zout_dr[:].opt()])
                nc.sync.dma_start(out=zar_sb[:], in_=zout_dr[:])
                nc.vector.tensor_tensor(out=zar_sb[:], in0=zar_sb[:],
                                        in1=fc1bw_sb[:],
                                        op=mybir.AluOpType.add)
                pp = psP.tile([1, 1], f32, tag="pp")
                nc.tensor.matmul(out=pp[:], lhsT=zar_sb[:, 0:1],
                                 rhs=fc2w_sb[:, 0:1], start=True, stop=False)
                nc.tensor.matmul(out=pp[:], lhsT=zar_sb[:, 1:2],
                                 rhs=fc2w_sb[:, 1:2], start=False, stop=True)
                nc.scalar.copy(out=pred_sb[:], in_=pp[:])
                nc.vector.tensor_tensor(out=pred_sb[:], in0=pred_sb[:],
                                        in1=fc2b_sb[:],
                                        op=mybir.AluOpType.add)
                nc.sync.dma_start(out=out_d.ap(), in_=pred_sb[:])

    nc.compile()
    return nc


# ------------------------------------------------------------- host glue ---
def make_in_maps(pl, inputs):
    import ml_dtypes
    bf = ml_dtypes.bfloat16
    x = np.asarray(inputs["x"], np.float32)
    W1l = np.asarray(inputs["W1l"], np.float32)
    b1l = np.asarray(inputs["b1l"], np.float32)
    W1r = np.asarray(inputs["W1r"], np.float32)
    W2l = np.asarray(inputs["W2l"], np.float32)
    W2r = np.asarray(inputs["W2r"], np.float32)
    fc1_W = np.asarray(inputs["fc1_W"], np.float32)
    fc1_b = np.asarray(inputs["fc1_b"], np.float32)
    fc2_W = np.asarray(inputs["fc2_W"], np.float32)

    iota = np.tile(np.arange(128, dtype=np.float32), (128, 1))
    eye = np.eye(128, dtype=np.float32)
    w1lT = np.ascontiguousarray(W1l.T).astype(bf)
    w1rT = np.ascontiguousarray(W1r.T).astype(bf)
    w2p = np.ascontiguousarray(np.stack([W2l[0], W2r[0]], axis=1)).astype(bf)
    b1 = np.ascontiguousarray(b1l.reshape(HID, 1))
    fc1bw = np.ascontiguousarray(fc1_b.reshape(2, 128).T)
    fc2w = np.ascontiguousarray(fc2_W[0].reshape(2, 128).T)

    in_maps = []
    for c in range(NCORES):
        p = pl["cores"][c]
        fc1T = np.zeros((NCPAD, LH), bf)
        fc1T[:NC] = fc1_W[:, c * NC:(c + 1) * NC].T.astype(bf)
        in_maps.append({
            "x_in": np.ascontiguousarray(x[c * NC:(c + 1) * NC]).astype(bf),
            "idxs": p["idxs"], "dst8": p["dst8"],
            "invrow": p["invrow"], "invw": np.ascontiguousarray(p["invw"]),
            "fc1T": fc1T,
            "w1lT": w1lT, "w1rT": w1rT, "b1": b1, "w2pair": w2p,
            "fc1bw": fc1bw, "fc2w": fc2w,
            "iota": iota, "id128b": eye.astype(bf), "id128f": eye,
        })
    return in_maps


def kernel(**inputs) -> np.ndarray:
    from concourse.bass_utils import run_bass_kernel_spmd
    pl = plan(np.asarray(inputs["edge_index"]))
    nc = build_bass(pl["H"],
                    b2val=float(np.asarray(inputs["b2l"]).reshape(-1)[0]),
                    fc2bval=float(np.asarray(inputs["fc2_b"]).reshape(-1)[0]))
    in_maps = make_in_maps(pl, inputs)
    res = run_bass_kernel_spmd(nc, in_maps, core_ids=list(range(NCORES)))
    pred = np.asarray(res.results[0]["out"], np.float32).reshape(())
    return pred


# revision 7
# speedup vs baseline: 4.9437x; 1.5895x over previous
"""Trainium2 Bass kernel for nn_GCNModel_75874892251953 (2-layer SAGEConv GNN
+ fc head), distributed over 8 NeuronCores.

Two cost facts drive the design:
 - The axon host->device tunnel moves ~36 MB/s shared across all 8 cores,
   so uploaded bytes are minimized (x sharded bf16 + device AllGather, fc1
   in bf16, int16 edge indices replicated on device).
 - This runtime dispatches instructions at ~40 us each, so per-edge work
   uses the batched Q7 ucode primitives dma_gather / dma_scatter_add
   (one instruction per up to 3072 edges) instead of per-tile one-hot
   matmuls.

Aggregation (both layers) = gather rows by src + scatter-add rows by dst
into a DRAM table. dma_scatter_add loses colliding updates, so edges are
batched by rank-within-destination: within a batch every dst is unique.
Gather indices are int16, so each batch is split into lo/hi halves of the
source table (<32768 rows each). Pad tokens gather row 0 and scatter into
a dump row. Node tables use the shard-padded layout ([8 x 6272, 64]) so
one index array serves both the x table (layer 1) and the s table
(layer 2, column 0 = s, built by SBUF expansion + AllGather).
"""
import numpy as np

NCORES = 8
N = 50000
IN = 64
HID = 128
LH = 256
NC = N // NCORES          # 6250 nodes per core
CH = -(-NC // 128)        # 49 chunks of 128 dst slots
NCPAD = CH * 128          # 6272
NCL = NC - 128 * (CH - 1)  # rows in the last partial chunk = 106
NPAD = NCORES * NCPAD     # padded global rows = 50176
NHALF = NPAD // 2         # 25088, gather-window split (int16 indices)
NBCAP = 24                # max 128-token blocks per gather/scatter


# --------------------------------------------------------------- planner ---
def plan(edge_index):
    src = np.asarray(edge_index[0], dtype=np.int64)
    dst = np.asarray(edge_index[1], dtype=np.int64)
    owner = dst // NC
    gsrc = (src // NC) * NCPAD + (src % NC)       # padded global row

    cores = []
    R = 1
    for c in range(NCORES):
        m = owner == c
        s_c = gsrc[m]
        d_c = dst[m] - c * NC
        order = np.argsort(d_c, kind="stable")
        s_c, d_c = s_c[order], d_c[order]
        deg = np.bincount(d_c, minlength=NC)
        R = max(R, int(deg.max()))
        starts = np.concatenate([[0], np.cumsum(deg)])[:-1]
        rank = np.arange(d_c.size) - starts[d_c]
        cores.append((s_c, d_c, rank, deg))

    # global (SPMD-uniform) sub-batch sizes: rank x part, split at NBCAP
    counts = np.zeros((NCORES, R, 2), np.int64)
    for c in range(NCORES):
        s_c, d_c, rank, deg = cores[c]
        hi = (s_c >= NHALF).astype(np.int64)
        np.add.at(counts[c], (rank, hi), 1)
    maxcnt = counts.max(axis=0)                   # [R, 2]

    batches = []            # (r, part, col0, nb, tok0_within_(r,part))
    col = 0
    for r in range(R):
        for part in (0, 1):
            nb_total = -(-int(maxcnt[r, part]) // 128)
            t0 = 0
            while nb_total > 0:
                nb = min(nb_total, NBCAP)
                batches.append((r, part, col, nb, t0))
                col += nb * 8
                t0 += nb * 128
                nb_total -= nb
    WT = col
    TT = WT * 16

    percore = []
    for c in range(NCORES):
        s_c, d_c, rank, deg = cores[c]
        hi = s_c >= NHALF
        gtok = np.zeros(TT, np.int64)
        stok = np.full(TT, NCPAD, np.int64)       # pad -> dump row
        # place each (r, part)'s edges contiguously across its sub-batches
        bypart = {}
        for (r, part, col0, nb, t0) in batches:
            bypart.setdefault((r, part), []).append((col0, nb, t0))
        for (r, part), subs in bypart.items():
            sel = (rank == r) & (hi if part else ~hi)
            gs = s_c[sel] - (NHALF if part else 0)
            ds = d_c[sel]
            n = gs.size
            for (col0, nb, t0) in subs:
                lo_t = t0
                hi_t = min(t0 + nb * 128, n)
                if hi_t <= lo_t:
                    continue
                pos = col0 * 16
                cnt = hi_t - lo_t
                gtok[pos:pos + cnt] = gs[lo_t:hi_t]
                stok[pos:pos + cnt] = ds[lo_t:hi_t]
        invd = (1.0 / np.maximum(deg, 1.0)).astype(np.float32)
        invrow = np.concatenate([invd, np.ones(NCPAD - NC, np.float32)])
        percore.append({
            "gidx": gtok.reshape(TT // 16, 16).T.astype(np.int16).copy(),
            "sidx": stok.reshape(TT // 16, 16).T.astype(np.int16).copy(),
            "invw": invrow.reshape(CH, 128).T.copy(),
        })
    return {"R": R, "WT": WT, "batches": batches, "cores": percore}


# ----------------------------------------------------------- bass builder ---
def build_bass(WT, batches, b2val=0.0, fc2bval=0.0):
    import concourse.bacc as bacc
    import concourse.tile as tile
    import concourse.mybir as mybir

    f32 = mybir.dt.float32
    bf16 = mybir.dt.bfloat16
    i16 = mybir.dt.int16

    NBMAX = max(nb for _, _, _, nb, _ in batches)

    nc = bacc.Bacc("TRN2", target_bir_lowering=False, debug=False,
                   num_devices=NCORES)

    def din(name, shape, dt=f32):
        return nc.dram_tensor(name, shape, dt, kind="ExternalInput")

    x_in_d = din("x_in", [NCPAD, IN], bf16)
    gidx_d = din("gidx", [16, WT], i16)
    sidx_d = din("sidx", [16, WT], i16)
    invw_d = din("invw", [128, CH])
    fc1T_d = din("fc1T", [NCPAD, LH], bf16)
    w1lT_d = din("w1lT", [IN, HID], bf16)
    w1rT_d = din("w1rT", [IN, HID], bf16)
    b1_d = din("b1", [HID, 1])
    w2p_d = din("w2pair", [HID, 2], bf16)
    fc1bw_d = din("fc1bw", [128, 2])
    fc2w_d = din("fc2w", [128, 2])
    idb_d = din("id128b", [128, 128], bf16)
    idf_d = din("id128f", [128, 128])
    out_d = nc.dram_tensor("out", [1, 1], f32, kind="ExternalOutput")

    RG = [list(range(NCORES))]

    with tile.TileContext(nc) as tc:
        with (
            tc.tile_pool(name="const", bufs=1) as cpool,
            tc.tile_pool(name="big", bufs=1) as bigpool,
            tc.tile_pool(name="fc1w", bufs=1) as fc1pool,
            tc.tile_pool(name="gbuf", bufs=3) as gpool,
            tc.tile_pool(name="dram", bufs=1, space="DRAM") as dpool,
        ):
            def load(pool, dram, shape, dt=f32):
                t = pool.tile(shape, dt, tag=dram.name + "_sb")
                nc.sync.dma_start(out=t[:], in_=dram.ap())
                return t

            # edge index arrays, replicated to the 8 Q7-core stripes
            gidx_sb = bigpool.tile([128, WT], i16, tag="gidx")
            sidx_sb = bigpool.tile([128, WT], i16, tag="sidx")
            for b in range(8):
                nc.sync.dma_start(out=gidx_sb[16 * b:16 * (b + 1), :],
                                  in_=gidx_d.ap())
                nc.sync.dma_start(out=sidx_sb[16 * b:16 * (b + 1), :],
                                  in_=sidx_d.ap())

            invw_sb = load(cpool, invw_d, [128, CH])
            idb_sb = load(cpool, idb_d, [128, 128], bf16)
            idf_sb = load(cpool, idf_d, [128, 128])
            w1lT_sb = load(cpool, w1lT_d, [IN, HID], bf16)
            w1rT_sb = load(cpool, w1rT_d, [IN, HID], bf16)
            b1_sb = load(cpool, b1_d, [HID, 1])
            w2p_sb = load(cpool, w2p_d, [HID, 2], bf16)
            fc1bw_sb = load(cpool, fc1bw_d, [128, 2])
            fc2w_sb = load(cpool, fc2w_d, [128, 2])

            fc1_tiles = []
            for k in range(CH):
                t = fc1pool.tile([128, LH], bf16, tag=f"fc1w{k}")
                nc.sync.dma_start(out=t[:],
                                  in_=fc1T_d.ap()[k * 128:(k + 1) * 128, :])
                fc1_tiles.append(t)

            srw_sb = bigpool.tile([128, 2 * CH], f32, tag="srw")
            zs_sb = bigpool.tile([128, 784], f32, tag="zs")
            nc.vector.memset(zs_sb[:], 0.0)
            b2rep_sb = cpool.tile([128, 1], f32, tag="b2rep")
            nc.vector.memset(b2rep_sb[:], b2val)
            fc2b_sb = cpool.tile([1, 1], f32, tag="fc2brep")
            nc.vector.memset(fc2b_sb[:], fc2bval)
            pred_sb = cpool.tile([1, 1], f32, tag="pred")
            zar_sb = cpool.tile([128, 2], f32, tag="zar")
            zf_sb = cpool.tile([128, 2], f32, tag="zf")

            # ---- stage own x shard to DRAM (f32), AllGather to full table
            xsb = bigpool.tile([128, NCPAD * IN // 128], f32, tag="xstg")
            nc.gpsimd.dma_start(out=xsb[:], in_=x_in_d.ap())   # bf16 -> f32
            x_stage = dpool.tile([1, NCPAD * IN], f32)
            nc.sync.dma_start(out=x_stage[:], in_=xsb[:])
            x_full = dpool.tile([NPAD * IN, 1], f32)
            eu_stage = dpool.tile([1, NCPAD * IN], f32)
            us_full = dpool.tile([NPAD * IN, 1], f32)
            agg_tbl = dpool.tile([NCPAD + 1, IN], f32)
            q_tbl = dpool.tile([NCPAD + 1, IN], f32)
            zin_dr = dpool.tile([128, 2], f32)
            zout_dr = dpool.tile([128, 2], f32)

            nc.gpsimd.collective_compute(
                "AllGather", mybir.AluOpType.bypass, replica_groups=RG,
                ins=[x_stage[:].opt()], outs=[x_full[:].opt()])
            x_lo = x_full[0:NHALF * IN, :].rearrange("(n f) 1 -> n f", f=IN)
            x_hi = x_full[NHALF * IN:NPAD * IN, :].rearrange(
                "(n f) 1 -> n f", f=IN)

            # zero the aggregation tables (incl. dump row)
            for t in (agg_tbl, q_tbl):
                for piece in range(4):
                    nc.sync.dma_start(
                        out=t[piece * 1568:(piece + 1) * 1568, :],
                        in_=zs_sb[:])
                nc.sync.dma_start(out=t[NCPAD:NCPAD + 1, :],
                                  in_=zs_sb[0:1, 0:64])

            # =================== layer-1 edge aggregation ===================
            for (r, part, col0, nb, t0) in batches:
                gt = gpool.tile([128, NBMAX, IN], f32, tag="g1")
                win = x_lo if part == 0 else x_hi
                nc.gpsimd.dma_gather(
                    out_ap=gt[:, 0:nb, :], in_ap=win,
                    idxs_ap=gidx_sb[:, col0:col0 + nb * 8],
                    num_idxs=nb * 128, num_idxs_reg=nb * 128, elem_size=IN,
                    single_packet=False)
                nc.gpsimd.dma_scatter_add(
                    out_ap=agg_tbl[:], in_ap=gt[:, 0:nb, :],
                    idxs_ap=sidx_sb[:, col0:col0 + nb * 8],
                    num_idxs=nb * 128, num_idxs_reg=nb * 128, elem_size=IN,
                    single_packet=False)

            # =================== per-chunk h1 / s / r ===================
            aggsb = bigpool.tile([128, CH, IN], f32, tag="aggsb")
            nc.sync.dma_start(
                out=aggsb[:],
                in_=agg_tbl[0:NCPAD, :].rearrange("(k p) f -> p k f", p=128))
            xcs = bigpool.tile([128, CH, IN], bf16, tag="xcs")
            nc.sync.dma_start(
                out=xcs[:],
                in_=x_in_d.ap().rearrange("(k p) f -> p k f", p=128))

            with (
                tc.tile_pool(name="psA", bufs=2, space="PSUM") as psA,
                tc.tile_pool(name="psX", bufs=2, space="PSUM") as psX,
                tc.tile_pool(name="psH", bufs=2, space="PSUM") as psH,
                tc.tile_pool(name="psSR", bufs=1, space="PSUM") as psSR,
                tc.tile_pool(name="chp", bufs=3) as chpool,
            ):
                for k in range(CH):
                    scb = chpool.tile([128, IN], bf16, tag="scb")
                    nc.vector.tensor_scalar(
                        out=scb[:], in0=aggsb[:, k, :],
                        scalar1=invw_sb[:, k:k + 1], scalar2=None,
                        op0=mybir.AluOpType.mult)
                    pxA = psA.tile([IN, 128], bf16, tag="pxA")
                    nc.tensor.transpose(out=pxA[:], in_=scb[:],
                                        identity=idb_sb[:])
                    aggT = chpool.tile([IN, 128], bf16, tag="aggT")
                    nc.vector.tensor_copy(out=aggT[:], in_=pxA[:])
                    pxX = psX.tile([IN, 128], bf16, tag="pxX")
                    nc.tensor.transpose(out=pxX[:], in_=xcs[:, k, :],
                                        identity=idb_sb[:])
                    xT = chpool.tile([IN, 128], bf16, tag="xT")
                    nc.vector.tensor_copy(out=xT[:], in_=pxX[:])
                    ph = psH.tile([HID, 128], f32, tag="psH")
                    nc.tensor.matmul(out=ph[:], lhsT=w1lT_sb[:], rhs=aggT[:],
                                     start=True, stop=False)
                    nc.tensor.matmul(out=ph[:], lhsT=w1rT_sb[:], rhs=xT[:],
                                     start=False, stop=True)
                    h1c = chpool.tile([HID, 128], bf16, tag="h1c")
                    nc.scalar.activation(
                        out=h1c[:], in_=ph[:],
                        func=mybir.ActivationFunctionType.Relu,
                        bias=b1_sb[:, 0:1])
                    psr = psSR.tile([128, 2], f32, tag="psSR")
                    nc.tensor.matmul(out=psr[:], lhsT=h1c[:], rhs=w2p_sb[:],
                                     start=True, stop=True)
                    nc.scalar.copy(out=srw_sb[:, 2 * k:2 * k + 2],
                                   in_=psr[:])

            # s table shard: rows [s, 0, ..., 0] via SBUF expansion.
            # exp flattens partition-major (row = p*CH+k), so s must be
            # reloaded in the matching [128, CH] row-major linear layout.
            with tc.tile_pool(name="psST", bufs=1, space="PSUM") as psST:
                pst = psST.tile([CH, 128], f32, tag="pst")
                nc.tensor.transpose(out=pst[:], in_=srw_sb[:, 0:2 * CH:2],
                                    identity=idf_sb[:])
                s_cm = bigpool.tile([CH, 128], f32, tag="s_cm")
                nc.vector.tensor_copy(out=s_cm[:], in_=pst[:])
            s_lin = dpool.tile([1, NCPAD], f32)
            nc.sync.dma_start(out=s_lin[:], in_=s_cm[:])
            s_pk = bigpool.tile([128, CH], f32, tag="s_pk")
            nc.sync.dma_start(out=s_pk[:], in_=s_lin[:])
            exp_sb = bigpool.tile([128, CH, IN], f32, tag="exp")
            nc.vector.memset(exp_sb[:], 0.0)
            nc.vector.tensor_copy(out=exp_sb[:, :, 0:1],
                                  in_=s_pk[:].unsqueeze(2))
            nc.sync.dma_start(out=eu_stage[:], in_=exp_sb[:])
            nc.gpsimd.collective_compute(
                "AllGather", mybir.AluOpType.bypass, replica_groups=RG,
                ins=[eu_stage[:].opt()], outs=[us_full[:].opt()])
            us_lo = us_full[0:NHALF * IN, :].rearrange("(n f) 1 -> n f", f=IN)
            us_hi = us_full[NHALF * IN:NPAD * IN, :].rearrange(
                "(n f) 1 -> n f", f=IN)

            # =================== layer-2 edge aggregation ===================
            for (r, part, col0, nb, t0) in batches:
                gt = gpool.tile([128, NBMAX, IN], f32, tag="g2")
                win = us_lo if part == 0 else us_hi
                nc.gpsimd.dma_gather(
                    out_ap=gt[:, 0:nb, :], in_ap=win,
                    idxs_ap=gidx_sb[:, col0:col0 + nb * 8],
                    num_idxs=nb * 128, num_idxs_reg=nb * 128, elem_size=IN,
                    single_packet=False)
                nc.gpsimd.dma_scatter_add(
                    out_ap=q_tbl[:], in_ap=gt[:, 0:nb, :],
                    idxs_ap=sidx_sb[:, col0:col0 + nb * 8],
                    num_idxs=nb * 128, num_idxs_reg=nb * 128, elem_size=IN,
                    single_packet=False)

            # =================== head ===================
            with (
                tc.tile_pool(name="psZ", bufs=1, space="PSUM") as psZ,
                tc.tile_pool(name="psP", bufs=1, space="PSUM") as psP,
            ):
                qrows = bigpool.tile([128, CH, IN], f32, tag="qrows")
                nc.sync.dma_start(
                    out=qrows[:],
                    in_=q_tbl[0:NCPAD, :].rearrange("(k p) f -> p k f",
                                                    p=128))
                vw_sb = cpool.tile([128, CH], f32, tag="vw")
                nc.vector.tensor_copy(out=vw_sb[:].unsqueeze(2),
                                      in_=qrows[:, :, 0:1])
                nc.vector.tensor_tensor(out=vw_sb[:], in0=vw_sb[:],
                                        in1=invw_sb[:],
                                        op=mybir.AluOpType.mult)
                nc.vector.tensor_tensor(out=vw_sb[:], in0=vw_sb[:],
                                        in1=srw_sb[:, 1:2 * CH:2],
                                        op=mybir.AluOpType.add)
                nc.scalar.activation(out=vw_sb[:], in_=vw_sb[:],
                                     func=mybir.ActivationFunctionType.Relu,
                                     bias=b2rep_sb[:, 0:1])
                vbf_sb = cpool.tile([128, CH], bf16, tag="vbf")
                nc.vector.tensor_copy(out=vbf_sb[:], in_=vw_sb[:])

                pz0 = psZ.tile([128, 1], f32, tag="pz0")
                pz1 = psZ.tile([128, 1], f32, tag="pz1")
                for k in range(CH):
                    nc.tensor.matmul(out=pz0[:],
                                     lhsT=fc1_tiles[k][:, 0:128],
                                     rhs=vbf_sb[:, k:k + 1],
                                     start=(k == 0), stop=(k == CH - 1))
                    nc.tensor.matmul(out=pz1[:],
                                     lhsT=fc1_tiles[k][:, 128:LH],
                                     rhs=vbf_sb[:, k:k + 1],
                                     start=(k == 0), stop=(k == CH - 1))
                nc.scalar.copy(out=zf_sb[:, 0:1], in_=pz0[:])
                nc.scalar.copy(out=zf_sb[:, 1:2], in_=pz1[:])
                nc.sync.dma_start(out=zin_dr[:], in_=zf_sb[:])
                nc.gpsimd.collective_compute(
                    "AllReduce", mybir.AluOpType.add, replica_groups=RG,
                    ins=[zin_dr[:].opt()], outs=[zout_dr[:].opt()])
                nc.sync.dma_start(out=zar_sb[:], in_=zout_dr[:])
                nc.vector.tensor_tensor(out=zar_sb[:], in0=zar_sb[:],
                                        in1=fc1bw_sb[:],
                                        op=mybir.AluOpType.add)
                pp = psP.tile([1, 1], f32, tag="pp")
                nc.tensor.matmul(out=pp[:], lhsT=zar_sb[:, 0:1],
                                 rhs=fc2w_sb[:, 0:1], start=True, stop=False)
                nc.tensor.matmul(out=pp[:], lhsT=zar_sb[:, 1:2],
                                 rhs=fc2w_sb[:, 1:2], start=False, stop=True)
                nc.scalar.copy(out=pred_sb[:], in_=pp[:])
                nc.vector.tensor_tensor(out=pred_sb[:], in0=pred_sb[:],
                                        in1=fc2b_sb[:],
                                        op=mybir.AluOpType.add)
                nc.sync.dma_start(out=out_d.ap(), in_=pred_sb[:])

    nc.compile()
    return nc


# ------------------------------------------------------------- host glue ---
def make_in_maps(pl, inputs):
    import ml_dtypes
    bf = ml_dtypes.bfloat16
    x = np.asarray(inputs["x"], np.float32)
    W1l = np.asarray(inputs["W1l"], np.float32)
    b1l = np.asarray(inputs["b1l"], np.float32)
    W1r = np.asarray(inputs["W1r"], np.float32)
    W2l = np.asarray(inputs["W2l"], np.float32)
    W2r = np.asarray(inputs["W2r"], np.float32)
    fc1_W = np.asarray(inputs["fc1_W"], np.float32)
    fc1_b = np.asarray(inputs["fc1_b"], np.float32)
    fc2_W = np.asarray(inputs["fc2_W"], np.float32)

    eye = np.eye(128, dtype=np.float32)
    w1lT = np.ascontiguousarray(W1l.T).astype(bf)
    w1rT = np.ascontiguousarray(W1r.T).astype(bf)
    w2p = np.ascontiguousarray(np.stack([W2l[0], W2r[0]], axis=1)).astype(bf)
    b1 = np.ascontiguousarray(b1l.reshape(HID, 1))
    fc1bw = np.ascontiguousarray(fc1_b.reshape(2, 128).T)
    fc2w = np.ascontiguousarray(fc2_W[0].reshape(2, 128).T)

    in_maps = []
    for c in range(NCORES):
        p = pl["cores"][c]
        xc = np.zeros((NCPAD, IN), bf)
        xc[:NC] = x[c * NC:(c + 1) * NC].astype(bf)
        fc1T = np.zeros((NCPAD, LH), bf)
        fc1T[:NC] = fc1_W[:, c * NC:(c + 1) * NC].T.astype(bf)
        in_maps.append({
            "x_in": xc,
            "gidx": p["gidx"], "sidx": p["sidx"],
            "invw": np.ascontiguousarray(p["invw"]),
            "fc1T": fc1T,
            "w1lT": w1lT, "w1rT": w1rT, "b1": b1, "w2pair": w2p,
            "fc1bw": fc1bw, "fc2w": fc2w,
            "id128b": eye.astype(bf), "id128f": eye,
        })
    return in_maps


def kernel(**inputs) -> np.ndarray:
    from concourse.bass_utils import run_bass_kernel_spmd
    pl = plan(np.asarray(inputs["edge_index"]))
    nc = build_bass(pl["WT"], pl["batches"],
                    b2val=float(np.asarray(inputs["b2l"]).reshape(-1)[0]),
                    fc2bval=float(np.asarray(inputs["fc2_b"]).reshape(-1)[0]))
    in_maps = make_in_maps(pl, inputs)
    res = run_bass_kernel_spmd(nc, in_maps, core_ids=list(range(NCORES)))
    pred = np.asarray(res.results[0]["out"], np.float32).reshape(())
    return pred


# revision 10
# speedup vs baseline: 10.0547x; 2.0339x over previous
"""Trainium2 Bass kernel for nn_GCNModel_75874892251953 (2-layer SAGEConv GNN
+ fc head), distributed over 8 NeuronCores.

Two cost facts drive the design:
 - The axon host->device tunnel moves ~36 MB/s shared across all 8 cores,
   so uploaded bytes are minimized (x sharded bf16 + device AllGather, fc1
   in bf16, int16 edge indices replicated on device).
 - This runtime dispatches instructions at ~40 us each, so per-edge work
   uses the batched Q7 ucode primitives dma_gather / dma_scatter_add
   (one instruction per up to 3072 edges) instead of per-tile one-hot
   matmuls.

Aggregation (both layers) = gather rows by src + scatter-add rows by dst
into a DRAM table. dma_scatter_add loses colliding updates, so edges are
batched by rank-within-destination: within a batch every dst is unique.
Gather indices are int16, so each batch is split into lo/hi halves of the
source table (<32768 rows each). Pad tokens gather row 0 and scatter into
a dump row. Node tables use the shard-padded layout ([8 x 6272, 64]) so
one index array serves both the x table (layer 1) and the s table
(layer 2, column 0 = s, built by SBUF expansion + AllGather).
"""
import numpy as np

NCORES = 8
N = 50000
IN = 64
HID = 128
LH = 256
NC = N // NCORES          # 6250 nodes per core
CH = -(-NC // 128)        # 49 chunks of 128 dst slots
NCPAD = CH * 128          # 6272
NCL = NC - 128 * (CH - 1)  # rows in the last partial chunk = 106
NPAD = NCORES * NCPAD     # padded global rows = 50176
NHALF = NPAD // 2         # 25088, gather-window split (int16 indices)
NBCAP = 24                # max 128-token blocks per gather/scatter


# --------------------------------------------------------------- planner ---
def plan(edge_index):
    src = np.asarray(edge_index[0], dtype=np.int64)
    dst = np.asarray(edge_index[1], dtype=np.int64)
    owner = dst // NC
    gsrc = (src // NC) * NCPAD + (src % NC)       # padded global row

    cores = []
    R = 1
    for c in range(NCORES):
        m = owner == c
        s_c = gsrc[m]
        d_c = dst[m] - c * NC
        order = np.argsort(d_c, kind="stable")
        s_c, d_c = s_c[order], d_c[order]
        deg = np.bincount(d_c, minlength=NC)
        R = max(R, int(deg.max()))
        starts = np.concatenate([[0], np.cumsum(deg)])[:-1]
        rank = np.arange(d_c.size) - starts[d_c]
        cores.append((s_c, d_c, rank, deg))

    # global (SPMD-uniform) sub-batch sizes: rank x part, split at NBCAP
    counts = np.zeros((NCORES, R, 2), np.int64)
    for c in range(NCORES):
        s_c, d_c, rank, deg = cores[c]
        hi = (s_c >= NHALF).astype(np.int64)
        np.add.at(counts[c], (rank, hi), 1)
    maxcnt = counts.max(axis=0)                   # [R, 2]

    batches = []            # (r, part, col0, nb, tok0_within_(r,part))
    col = 0
    for r in range(R):
        for part in (0, 1):
            nb_total = -(-int(maxcnt[r, part]) // 128)
            t0 = 0
            while nb_total > 0:
                nb = min(nb_total, NBCAP)
                batches.append((r, part, col, nb, t0))
                col += nb * 8
                t0 += nb * 128
                nb_total -= nb
    WT = col
    TT = WT * 16

    percore = []
    for c in range(NCORES):
        s_c, d_c, rank, deg = cores[c]
        hi = s_c >= NHALF
        gtok = np.zeros(TT, np.int64)
        stok = np.full(TT, NCPAD, np.int64)       # pad -> dump row
        # place each (r, part)'s edges contiguously across its sub-batches
        bypart = {}
        for (r, part, col0, nb, t0) in batches:
            bypart.setdefault((r, part), []).append((col0, nb, t0))
        for (r, part), subs in bypart.items():
            sel = (rank == r) & (hi if part else ~hi)
            gs = s_c[sel] - (NHALF if part else 0)
            ds = d_c[sel]
            n = gs.size
            for (col0, nb, t0) in subs:
                lo_t = t0
                hi_t = min(t0 + nb * 128, n)
                if hi_t <= lo_t:
                    continue
                pos = col0 * 16
                cnt = hi_t - lo_t
                gtok[pos:pos + cnt] = gs[lo_t:hi_t]
                stok[pos:pos + cnt] = ds[lo_t:hi_t]
        invd = (1.0 / np.maximum(deg, 1.0)).astype(np.float32)
        invrow = np.concatenate([invd, np.ones(NCPAD - NC, np.float32)])
        percore.append({
            "gidx": gtok.reshape(TT // 16, 16).T.astype(np.int16).copy(),
            "sidx": stok.reshape(TT // 16, 16).T.astype(np.int16).copy(),
            "invw": invrow.reshape(CH, 128).T.copy(),
        })
    return {"R": R, "WT": WT, "batches": batches, "cores": percore}


# ----------------------------------------------------------- bass builder ---
def build_bass(WT, batches, b2val=0.0, tval=0.0):
    import concourse.bacc as bacc
    import concourse.tile as tile
    import concourse.mybir as mybir

    f32 = mybir.dt.float32
    bf16 = mybir.dt.bfloat16
    i16 = mybir.dt.int16

    NBMAX = max(nb for _, _, _, nb, _ in batches)

    nc = bacc.Bacc("TRN2", target_bir_lowering=False, debug=False,
                   num_devices=NCORES)

    def din(name, shape, dt=f32):
        return nc.dram_tensor(name, shape, dt, kind="ExternalInput")

    x_in_d = din("x_in", [NCPAD, IN], bf16)
    gidx_d = din("gidx", [16, WT], i16)
    sidx_d = din("sidx", [16, WT], i16)
    invw_d = din("invw", [128, CH])
    cw_d = din("cw", [128, CH])
    w1lT_d = din("w1lT", [IN, HID], bf16)
    w1rT_d = din("w1rT", [IN, HID], bf16)
    b1_d = din("b1", [HID, 1])
    w2p_d = din("w2pair", [HID, 2], bf16)
    idb_d = din("id128b", [128, 128], bf16)
    idf_d = din("id128f", [128, 128])
    out_d = nc.dram_tensor("out", [1, 1], f32, kind="ExternalOutput")

    RG = [list(range(NCORES))]

    with tile.TileContext(nc) as tc:
        with (
            tc.tile_pool(name="const", bufs=1) as cpool,
            tc.tile_pool(name="big", bufs=1) as bigpool,
            tc.tile_pool(name="gbuf", bufs=3) as gpool,
            tc.tile_pool(name="dram", bufs=1, space="DRAM") as dpool,
        ):
            def load(pool, dram, shape, dt=f32):
                t = pool.tile(shape, dt, tag=dram.name + "_sb")
                nc.sync.dma_start(out=t[:], in_=dram.ap())
                return t

            # edge index arrays, replicated to the 8 Q7-core stripes
            gidx_sb = bigpool.tile([128, WT], i16, tag="gidx")
            sidx_sb = bigpool.tile([128, WT], i16, tag="sidx")
            for b in range(8):
                nc.sync.dma_start(out=gidx_sb[16 * b:16 * (b + 1), :],
                                  in_=gidx_d.ap())
                nc.sync.dma_start(out=sidx_sb[16 * b:16 * (b + 1), :],
                                  in_=sidx_d.ap())

            invw_sb = load(cpool, invw_d, [128, CH])
            cw_sb = load(cpool, cw_d, [128, CH])
            idb_sb = load(cpool, idb_d, [128, 128], bf16)
            idf_sb = load(cpool, idf_d, [128, 128])
            w1lT_sb = load(cpool, w1lT_d, [IN, HID], bf16)
            w1rT_sb = load(cpool, w1rT_d, [IN, HID], bf16)
            b1_sb = load(cpool, b1_d, [HID, 1])
            w2p_sb = load(cpool, w2p_d, [HID, 2], bf16)

            srw_sb = bigpool.tile([128, 2 * CH], f32, tag="srw")
            zs_sb = bigpool.tile([128, 784], f32, tag="zs")
            nc.vector.memset(zs_sb[:], 0.0)
            b2rep_sb = cpool.tile([128, 1], f32, tag="b2rep")
            nc.vector.memset(b2rep_sb[:], b2val)
            tva_sb = cpool.tile([1, 1], f32, tag="tval")
            nc.vector.memset(tva_sb[:], tval)
            pred_sb = cpool.tile([1, 1], f32, tag="pred")
            zf_sb = cpool.tile([1, 8], f32, tag="zf")
            nc.vector.memset(zf_sb[:], 0.0)

            # ---- stage own x shard to DRAM (f32), AllGather to full table
            xsb = bigpool.tile([128, NCPAD * IN // 128], f32, tag="xstg")
            nc.gpsimd.dma_start(out=xsb[:], in_=x_in_d.ap())   # bf16 -> f32
            x_stage = dpool.tile([1, NCPAD * IN], f32)
            nc.sync.dma_start(out=x_stage[:], in_=xsb[:])
            x_full = dpool.tile([NPAD * IN, 1], f32)
            eu_stage = dpool.tile([1, NCPAD * IN], f32)
            us_full = dpool.tile([NPAD * IN, 1], f32)
            agg_tbl = dpool.tile([NCPAD + 1, IN], f32)
            q_tbl = dpool.tile([NCPAD + 1, IN], f32)
            zin_dr = dpool.tile([1, 8], f32)
            zout_dr = dpool.tile([1, 8], f32)

            nc.gpsimd.collective_compute(
                "AllGather", mybir.AluOpType.bypass, replica_groups=RG,
                ins=[x_stage[:].opt()], outs=[x_full[:].opt()])
            x_lo = x_full[0:NHALF * IN, :].rearrange("(n f) 1 -> n f", f=IN)
            x_hi = x_full[NHALF * IN:NPAD * IN, :].rearrange(
                "(n f) 1 -> n f", f=IN)

            # zero the aggregation tables (incl. dump row)
            for t in (agg_tbl, q_tbl):
                for piece in range(4):
                    nc.sync.dma_start(
                        out=t[piece * 1568:(piece + 1) * 1568, :],
                        in_=zs_sb[:])
                nc.sync.dma_start(out=t[NCPAD:NCPAD + 1, :],
                                  in_=zs_sb[0:1, 0:64])

            # =================== layer-1 edge aggregation ===================
            for (r, part, col0, nb, t0) in batches:
                gt = gpool.tile([128, NBMAX, IN], f32, tag="g1")
                win = x_lo if part == 0 else x_hi
                nc.gpsimd.dma_gather(
                    out_ap=gt[:, 0:nb, :], in_ap=win,
                    idxs_ap=gidx_sb[:, col0:col0 + nb * 8],
                    num_idxs=nb * 128, num_idxs_reg=nb * 128, elem_size=IN,
                    single_packet=False)
                nc.gpsimd.dma_scatter_add(
                    out_ap=agg_tbl[:], in_ap=gt[:, 0:nb, :],
                    idxs_ap=sidx_sb[:, col0:col0 + nb * 8],
                    num_idxs=nb * 128, num_idxs_reg=nb * 128, elem_size=IN,
                    single_packet=False)

            # =================== per-chunk h1 / s / r ===================
            aggsb = bigpool.tile([128, CH, IN], f32, tag="aggsb")
            nc.sync.dma_start(
                out=aggsb[:],
                in_=agg_tbl[0:NCPAD, :].rearrange("(k p) f -> p k f", p=128))
            xcs = bigpool.tile([128, CH, IN], bf16, tag="xcs")
            nc.sync.dma_start(
                out=xcs[:],
                in_=x_in_d.ap().rearrange("(k p) f -> p k f", p=128))

            with (
                tc.tile_pool(name="psA", bufs=2, space="PSUM") as psA,
                tc.tile_pool(name="psX", bufs=2, space="PSUM") as psX,
                tc.tile_pool(name="psH", bufs=2, space="PSUM") as psH,
                tc.tile_pool(name="psSR", bufs=1, space="PSUM") as psSR,
                tc.tile_pool(name="chp", bufs=3) as chpool,
            ):
                for k in range(CH):
                    scb = chpool.tile([128, IN], bf16, tag="scb")
                    nc.vector.tensor_scalar(
                        out=scb[:], in0=aggsb[:, k, :],
                        scalar1=invw_sb[:, k:k + 1], scalar2=None,
                        op0=mybir.AluOpType.mult)
                    pxA = psA.tile([IN, 128], bf16, tag="pxA")
                    nc.tensor.transpose(out=pxA[:], in_=scb[:],
                                        identity=idb_sb[:])
                    aggT = chpool.tile([IN, 128], bf16, tag="aggT")
                    nc.vector.tensor_copy(out=aggT[:], in_=pxA[:])
                    pxX = psX.tile([IN, 128], bf16, tag="pxX")
                    nc.tensor.transpose(out=pxX[:], in_=xcs[:, k, :],
                                        identity=idb_sb[:])
                    xT = chpool.tile([IN, 128], bf16, tag="xT")
                    nc.vector.tensor_copy(out=xT[:], in_=pxX[:])
                    ph = psH.tile([HID, 128], f32, tag="psH")
                    nc.tensor.matmul(out=ph[:], lhsT=w1lT_sb[:], rhs=aggT[:],
                                     start=True, stop=False)
                    nc.tensor.matmul(out=ph[:], lhsT=w1rT_sb[:], rhs=xT[:],
                                     start=False, stop=True)
                    h1c = chpool.tile([HID, 128], bf16, tag="h1c")
                    nc.scalar.activation(
                        out=h1c[:], in_=ph[:],
                        func=mybir.ActivationFunctionType.Relu,
                        bias=b1_sb[:, 0:1])
                    psr = psSR.tile([128, 2], f32, tag="psSR")
                    nc.tensor.matmul(out=psr[:], lhsT=h1c[:], rhs=w2p_sb[:],
                                     start=True, stop=True)
                    nc.scalar.copy(out=srw_sb[:, 2 * k:2 * k + 2],
                                   in_=psr[:])

            # s table shard: rows [s, 0, ..., 0] via SBUF expansion.
            # exp flattens partition-major (row = p*CH+k), so s must be
            # reloaded in the matching [128, CH] row-major linear layout.
            with tc.tile_pool(name="psST", bufs=1, space="PSUM") as psST:
                pst = psST.tile([CH, 128], f32, tag="pst")
                nc.tensor.transpose(out=pst[:], in_=srw_sb[:, 0:2 * CH:2],
                                    identity=idf_sb[:])
                s_cm = bigpool.tile([CH, 128], f32, tag="s_cm")
                nc.vector.tensor_copy(out=s_cm[:], in_=pst[:])
            s_lin = dpool.tile([1, NCPAD], f32)
            nc.sync.dma_start(out=s_lin[:], in_=s_cm[:])
            s_pk = bigpool.tile([128, CH], f32, tag="s_pk")
            nc.sync.dma_start(out=s_pk[:], in_=s_lin[:])
            exp_sb = bigpool.tile([128, CH, IN], f32, tag="exp")
            nc.vector.memset(exp_sb[:], 0.0)
            nc.vector.tensor_copy(out=exp_sb[:, :, 0:1],
                                  in_=s_pk[:].unsqueeze(2))
            nc.sync.dma_start(out=eu_stage[:], in_=exp_sb[:])
            nc.gpsimd.collective_compute(
                "AllGather", mybir.AluOpType.bypass, replica_groups=RG,
                ins=[eu_stage[:].opt()], outs=[us_full[:].opt()])
            us_lo = us_full[0:NHALF * IN, :].rearrange("(n f) 1 -> n f", f=IN)
            us_hi = us_full[NHALF * IN:NPAD * IN, :].rearrange(
                "(n f) 1 -> n f", f=IN)

            # =================== layer-2 edge aggregation ===================
            for (r, part, col0, nb, t0) in batches:
                gt = gpool.tile([128, NBMAX, IN], f32, tag="g2")
                win = us_lo if part == 0 else us_hi
                nc.gpsimd.dma_gather(
                    out_ap=gt[:, 0:nb, :], in_ap=win,
                    idxs_ap=gidx_sb[:, col0:col0 + nb * 8],
                    num_idxs=nb * 128, num_idxs_reg=nb * 128, elem_size=IN,
                    single_packet=False)
                nc.gpsimd.dma_scatter_add(
                    out_ap=q_tbl[:], in_ap=gt[:, 0:nb, :],
                    idxs_ap=sidx_sb[:, col0:col0 + nb * 8],
                    num_idxs=nb * 128, num_idxs_reg=nb * 128, elem_size=IN,
                    single_packet=False)

            # =================== head ===================
            with (
                tc.tile_pool(name="psP", bufs=1, space="PSUM") as psP,
            ):
                qrows = bigpool.tile([128, CH, IN], f32, tag="qrows")
                nc.sync.dma_start(
                    out=qrows[:],
                    in_=q_tbl[0:NCPAD, :].rearrange("(k p) f -> p k f",
                                                    p=128))
                vw_sb = cpool.tile([128, CH], f32, tag="vw")
                nc.vector.tensor_copy(out=vw_sb[:].unsqueeze(2),
                                      in_=qrows[:, :, 0:1])
                nc.vector.tensor_tensor(out=vw_sb[:], in0=vw_sb[:],
                                        in1=invw_sb[:],
                                        op=mybir.AluOpType.mult)
                nc.vector.tensor_tensor(out=vw_sb[:], in0=vw_sb[:],
                                        in1=srw_sb[:, 1:2 * CH:2],
                                        op=mybir.AluOpType.add)
                nc.scalar.activation(out=vw_sb[:], in_=vw_sb[:],
                                     func=mybir.ActivationFunctionType.Relu,
                                     bias=b2rep_sb[:, 0:1])
                pp = psP.tile([1, 1], f32, tag="pp")
                for k in range(CH):
                    nc.tensor.matmul(out=pp[:], lhsT=cw_sb[:, k:k + 1],
                                     rhs=vw_sb[:, k:k + 1],
                                     start=(k == 0), stop=(k == CH - 1))
                nc.scalar.copy(out=zf_sb[:, 0:1], in_=pp[:])
                nc.sync.dma_start(out=zin_dr[:], in_=zf_sb[:])
                nc.gpsimd.collective_compute(
                    "AllReduce", mybir.AluOpType.add, replica_groups=RG,
                    ins=[zin_dr[:].opt()], outs=[zout_dr[:].opt()])
                zo_sb = cpool.tile([1, 8], f32, tag="zo")
                nc.sync.dma_start(out=zo_sb[:], in_=zout_dr[:])
                nc.vector.tensor_tensor(out=pred_sb[:], in0=zo_sb[:, 0:1],
                                        in1=tva_sb[:],
                                        op=mybir.AluOpType.add)
                nc.sync.dma_start(out=out_d.ap(), in_=pred_sb[:])

    nc.compile()
    return nc


# ------------------------------------------------------------- host glue ---
def make_in_maps(pl, inputs):
    import ml_dtypes
    bf = ml_dtypes.bfloat16
    x = np.asarray(inputs["x"], np.float32)
    W1l = np.asarray(inputs["W1l"], np.float32)
    b1l = np.asarray(inputs["b1l"], np.float32)
    W1r = np.asarray(inputs["W1r"], np.float32)
    W2l = np.asarray(inputs["W2l"], np.float32)
    W2r = np.asarray(inputs["W2r"], np.float32)
    fc1_W = np.asarray(inputs["fc1_W"], np.float32)
    fc1_b = np.asarray(inputs["fc1_b"], np.float32)
    fc2_W = np.asarray(inputs["fc2_W"], np.float32)

    eye = np.eye(128, dtype=np.float32)
    w1lT = np.ascontiguousarray(W1l.T).astype(bf)
    w1rT = np.ascontiguousarray(W1r.T).astype(bf)
    w2p = np.ascontiguousarray(np.stack([W2l[0], W2r[0]], axis=1)).astype(bf)
    b1 = np.ascontiguousarray(b1l.reshape(HID, 1))
    # fold the scalar head: pred = (fc2_W @ fc1_W) . v + fc2_W @ fc1_b + fc2_b
    cvec = (fc2_W[0].astype(np.float64) @ fc1_W.astype(np.float64)
            ).astype(np.float32)

    in_maps = []
    for c in range(NCORES):
        p = pl["cores"][c]
        xc = np.zeros((NCPAD, IN), bf)
        xc[:NC] = x[c * NC:(c + 1) * NC].astype(bf)
        cpad = np.zeros(NCPAD, np.float32)
        cpad[:NC] = cvec[c * NC:(c + 1) * NC]
        in_maps.append({
            "x_in": xc,
            "gidx": p["gidx"], "sidx": p["sidx"],
            "invw": np.ascontiguousarray(p["invw"]),
            "cw": np.ascontiguousarray(cpad.reshape(CH, 128).T),
            "w1lT": w1lT, "w1rT": w1rT, "b1": b1, "w2pair": w2p,
            "id128b": eye.astype(bf), "id128f": eye,
        })
    return in_maps


def kernel(**inputs) -> np.ndarray:
    from concourse.bass_utils import run_bass_kernel_spmd
    pl = plan(np.asarray(inputs["edge_index"]))
    fc2_W = np.asarray(inputs["fc2_W"], np.float64)
    fc1_b = np.asarray(inputs["fc1_b"], np.float64)
    tval = float(fc2_W[0] @ fc1_b
                 + np.asarray(inputs["fc2_b"], np.float64).reshape(-1)[0])
    nc = build_bass(pl["WT"], pl["batches"],
                    b2val=float(np.asarray(inputs["b2l"]).reshape(-1)[0]),
                    tval=tval)
    in_maps = make_in_maps(pl, inputs)
    res = run_bass_kernel_spmd(nc, in_maps, core_ids=list(range(NCORES)))
    pred = np.asarray(res.results[0]["out"], np.float32).reshape(())
    return pred
